# revision 3
# baseline (speedup 1.0000x reference)
"""Trainium2 Bass kernel for DynamicEdgeConstruction (top-k masked softmax
attention matrix).

Computes, for x [B=4, N=4096, C=256], W_q/W_k [256, 64]:
    Q = x @ W_q; K = x @ W_k
    S = Q K^T / sqrt(64)           [B, N, N]
    A = softmax over the top-k entries of each row of S, zeros elsewhere.

Sharding: 8 NeuronCores, 2 per batch element, each handling 2048 query rows
(row-wise sequence parallel; K replicated per batch).

Device algorithm per core (sparse formulation — output has only k nonzeros
per row):
  - S row-tile [128, 4096] via fp32 PE matmuls (scale folded into W_q host-side;
    1/8 is a power of two so this is bit-exact).
  - DVE max8 -> exact top-8 values per row; max_index -> their column indices.
  - Tiny softmax over the 8 values (ACT exp with per-row bias + accumulate).
  - Zero-fill the output tile via DMA; scatter the k values via indirect DMA.
"""

import numpy as np

B, N, C, DK = 4, 4096, 256, 64
NCORES = 8
RPC = N // 2          # rows per core (2048)
P = 128               # partitions
NT = RPC // P         # row tiles per core (16)
CHUNK = 512           # matmul free-dim chunk (one PSUM bank fp32)
HALF = 2048           # S half-tile free size (4 PSUM banks)

_cache = {}


def _build(k: int):
    """Build + compile the SPMD Bass program for top-k = k (1..8)."""
    import concourse.bass as bass
    import concourse.bacc as bacc
    import concourse.tile as tile
    import concourse.mybir as mybir
    from concourse.tile_rust import add_dep_helper
    from contextlib import ExitStack

    f32 = mybir.dt.float32
    u32 = mybir.dt.uint32

    nc = bacc.Bacc("TRN2", target_bir_lowering=False, debug=False,
                   num_devices=NCORES)

    xkT_d = nc.dram_tensor("xkT", [C, N], f32, kind="ExternalInput").ap()
    xqT_d = nc.dram_tensor("xqT", [C, RPC], f32, kind="ExternalInput").ap()
    wq_d = nc.dram_tensor("wq", [C, DK], f32, kind="ExternalInput").ap()
    wk_d = nc.dram_tensor("wk", [C, DK], f32, kind="ExternalInput").ap()
    out_d = nc.dram_tensor("out", [RPC * N, 1], f32, kind="ExternalOutput").ap()

    with tile.TileContext(nc) as tc:
        with ExitStack() as ctx:
            const = ctx.enter_context(tc.tile_pool(name="const", bufs=1))

            xkT = [const.tile([P, N], f32, tag=f"xkT{i}", name=f"xkT{i}")
                   for i in range(2)]
            xqT = [const.tile([P, RPC], f32, tag=f"xqT{i}", name=f"xqT{i}")
                   for i in range(2)]
            wq = [const.tile([P, DK], f32, tag=f"wq{i}", name=f"wq{i}")
                  for i in range(2)]
            wk = [const.tile([P, DK], f32, tag=f"wk{i}", name=f"wk{i}")
                  for i in range(2)]
            KT = const.tile([DK, N], f32, tag="KT")
            QT = const.tile([DK, RPC], f32, tag="QT")
            zero = const.tile([P, N], f32, tag="zero")

            for i in range(2):
                nc.sync.dma_start(xkT[i][:], xkT_d[i * P:(i + 1) * P, :])
                nc.sync.dma_start(xqT[i][:], xqT_d[i * P:(i + 1) * P, :])
                nc.sync.dma_start(wq[i][:], wq_d[i * P:(i + 1) * P, :])
                nc.sync.dma_start(wk[i][:], wk_d[i * P:(i + 1) * P, :])
            nc.vector.memset(zero[:], 0.0)

            # Zero-fill the whole output early; scatters depend on these.
            zdmas = []
            for t in range(NT):
                zdmas.append(nc.sync.dma_start(
                    out_d[t * P * N:(t + 1) * P * N, :], zero[:]))

            # Projections: KT = wk^T @ xkT, QT = wq^T @ xqT (contraction over
            # C = 256 in two accumulating halves).
            with tc.tile_pool(name="proj_ps", bufs=2, space="PSUM") as proj_ps:
                for ch in range(N // CHUNK):
                    pt = proj_ps.tile([DK, CHUNK], f32, tag="proj")
                    sl = slice(ch * CHUNK, (ch + 1) * CHUNK)
                    nc.tensor.matmul(out=pt[:], lhsT=wk[0][:], rhs=xkT[0][:, sl],
                                     start=True, stop=False)
                    nc.tensor.matmul(out=pt[:], lhsT=wk[1][:], rhs=xkT[1][:, sl],
                                     start=False, stop=True)
                    nc.scalar.copy(KT[:, sl], pt[:])
                for ch in range(RPC // CHUNK):
                    pt = proj_ps.tile([DK, CHUNK], f32, tag="proj")
                    sl = slice(ch * CHUNK, (ch + 1) * CHUNK)
                    nc.tensor.matmul(out=pt[:], lhsT=wq[0][:], rhs=xqT[0][:, sl],
                                     start=True, stop=False)
                    nc.tensor.matmul(out=pt[:], lhsT=wq[1][:], rhs=xqT[1][:, sl],
                                     start=False, stop=True)
                    nc.scalar.copy(QT[:, sl], pt[:])

            spool = ctx.enter_context(tc.tile_pool(name="ssb", bufs=2))
            small = ctx.enter_context(tc.tile_pool(name="small", bufs=3))
            sps = ctx.enter_context(tc.tile_pool(name="sps", bufs=2, space="PSUM"))

            for t in range(NT):
                s_sb = spool.tile([P, N], f32, tag="s_sb")
                lhsT = QT[:, t * P:(t + 1) * P]
                for h in range(2):
                    ps = sps.tile([P, HALF], f32, tag="sps")
                    for ch in range(HALF // CHUNK):
                        psl = slice(ch * CHUNK, (ch + 1) * CHUNK)
                        ksl = slice(h * HALF + ch * CHUNK,
                                    h * HALF + (ch + 1) * CHUNK)
                        nc.tensor.matmul(out=ps[:, psl], lhsT=lhsT,
                                         rhs=KT[:, ksl], start=True, stop=True)
                    nc.scalar.copy(s_sb[:, h * HALF:(h + 1) * HALF], ps[:])

                V = small.tile([P, 8], f32, tag="V")
                nc.vector.max(V[:], s_sb[:])
                idx = small.tile([P, 8], u32, tag="idx")
                nc.vector.max_index(idx[:], V[:], s_sb[:])

                negm = small.tile([P, 1], f32, tag="negm")
                nc.vector.tensor_scalar_mul(negm[:], V[:, 0:1], -1.0)
                if k < 8:
                    nc.vector.memset(V[:, k:8], -1e30)
                E8 = small.tile([P, 8], f32, tag="E8")
                Z = small.tile([P, 1], f32, tag="Z")
                nc.scalar.activation(E8[:], V[:], mybir.ActivationFunctionType.Exp,
                                     bias=negm[:, 0:1], scale=1.0, accum_out=Z[:])
                r = small.tile([P, 1], f32, tag="r")
                nc.vector.reciprocal(r[:], Z[:])
                A8 = small.tile([P, 8], f32, tag="A8")
                nc.vector.tensor_scalar_mul(A8[:], E8[:], r[:, 0:1])

                base = small.tile([P, 1], u32, tag="base")
                nc.gpsimd.iota(base[:], pattern=[[0, 1]], base=t * P * N,
                               channel_multiplier=N)
                off = small.tile([P, 8], u32, tag="off")
                nc.vector.tensor_tensor(off[:], idx[:],
                                        base[:, 0:1].to_broadcast([P, 8]),
                                        op=mybir.AluOpType.add)

                # HW indirect DMA consumes ONE offset per partition (writes the
                # whole free row contiguously at off[p, 0]) — so issue one
                # single-element scatter per top-k slot.
                for j in range(k):
                    sc = nc.gpsimd.indirect_dma_start(
                        out=out_d[:],
                        out_offset=bass.IndirectOffsetOnAxis(
                            ap=off[:, j:j + 1], axis=0),
                        in_=A8[:, j:j + 1],
                        in_offset=None,
                    )
                    add_dep_helper(sc.ins, zdmas[t].ins,
                                   reason="scatter after zero-fill of its tile")

    nc.compile()
    return nc


def _get_program(k: int):
    if k not in _cache:
        _cache[k] = _build(k)
    return _cache[k]


def kernel(x, W_q, W_k, top_k):
    from concourse.bass_utils import run_bass_kernel_spmd

    x = np.asarray(x, dtype=np.float32)
    W_q = np.asarray(W_q, dtype=np.float32)
    W_k = np.asarray(W_k, dtype=np.float32)
    k = int(np.asarray(top_k))
    assert x.shape == (B, N, C) and W_q.shape == (C, DK) and W_k.shape == (C, DK)
    assert 1 <= k <= 8, f"top_k={k} unsupported"

    nc = _get_program(k)

    wq_scaled = np.ascontiguousarray(W_q * np.float32(DK) ** np.float32(-0.5),
                                     dtype=np.float32)
    wk_c = np.ascontiguousarray(W_k, dtype=np.float32)

    in_maps = []
    for c in range(NCORES):
        b, half = c // 2, c % 2
        xT = np.ascontiguousarray(x[b].T)                      # [C, N]
        xqT = np.ascontiguousarray(xT[:, half * RPC:(half + 1) * RPC])
        in_maps.append({"xkT": xT, "xqT": xqT, "wq": wq_scaled, "wk": wk_c})

    res = run_bass_kernel_spmd(nc, in_maps, list(range(NCORES)))

    A = np.empty((B, N, N), dtype=np.float32)
    for c in range(NCORES):
        b, half = c // 2, c % 2
        A[b, half * RPC:(half + 1) * RPC, :] = res.results[c]["out"].reshape(RPC, N)
    return A


# revision 22
# speedup vs baseline: 3702.2183x; 3702.2183x over previous
"""Trainium2 Bass kernel for DynamicEdgeConstruction (top-k masked softmax
attention matrix).

Computes, for x [B=4, N=4096, C=256], W_q/W_k [256, 64]:
    Q = x @ W_q; K = x @ W_k
    S = Q K^T / sqrt(64)           [B, N, N]
    A = softmax over the top-k entries of each row of S, zeros elsewhere.

Sharding: 8 NeuronCores, 2 per batch element, each handling 2048 query rows
(row-wise sequence parallel; K replicated per batch).

Device algorithm per core (dense formulation):
  - S row-tile [128, 4096] via fp32 PE matmuls into PSUM (softmax scale folded
    into W_q host-side; 1/8 is a power of two so this is bit-exact).
  - ACT copies S to SBUF; DVE max8 gives the exact top-8 values per row.
  - Tiny softmax over the top-k values yields r = 1/Z per row.
  - maskr = (S >= t_k) * r  (one DVE tensor_scalar pass, per-row scalars).
  - E = exp(S - m)           (one ACT pass, per-row bias).
  - A = E * maskr            (tensor_tensor; split DVE/GpSimd for balance).
  - Plain DMA of the dense tile to the output.
"""

import numpy as np

B, N, C, DK = 4, 4096, 256, 64
NCORES = 8
RPC = N // 2          # rows per core (2048)
P = 128               # partitions
NT = RPC // P         # row tiles per core (16)
CHUNK = 512           # matmul free-dim chunk (one PSUM bank fp32)
HALF = 2048           # S half-tile free size (4 PSUM banks)

# which row-tiles run the final multiply on DVE (rest go to GpSimd)
DVE_MULT_TILES = frozenset()

# use float32r (fast fp32 mode) for PE matmuls
F32R = False

_cache = {}


def _build(k: int, f32r: bool = False):
    """Build + compile the SPMD Bass program for top-k = k (1..8)."""
    import concourse.bass as bass
    import concourse.bacc as bacc
    import concourse.tile as tile
    import concourse.mybir as mybir
    from contextlib import ExitStack

    f32 = mybir.dt.float32
    mmdt = (lambda ap: ap.bitcast(mybir.dt.float32r)) if f32r else (lambda ap: ap)

    nc = bacc.Bacc("TRN2", target_bir_lowering=False, debug=False,
                   num_devices=NCORES)

    xkT_d = nc.dram_tensor("xkT", [C, N], f32, kind="ExternalInput").ap()
    xqT_d = nc.dram_tensor("xqT", [C, RPC], f32, kind="ExternalInput").ap()
    wq_d = nc.dram_tensor("wq", [C, DK], f32, kind="ExternalInput").ap()
    wk_d = nc.dram_tensor("wk", [C, DK], f32, kind="ExternalInput").ap()
    out_d = nc.dram_tensor("out", [RPC, N], f32, kind="ExternalOutput").ap()

    with tile.TileContext(nc) as tc:
        with ExitStack() as ctx:
            const = ctx.enter_context(tc.tile_pool(name="const", bufs=1))

            xkT = [const.tile([P, N], f32, tag=f"xkT{i}", name=f"xkT{i}")
                   for i in range(2)]
            xqT = [const.tile([P, RPC], f32, tag=f"xqT{i}", name=f"xqT{i}")
                   for i in range(2)]
            wq = [const.tile([P, DK], f32, tag=f"wq{i}", name=f"wq{i}")
                  for i in range(2)]
            wk = [const.tile([P, DK], f32, tag=f"wk{i}", name=f"wk{i}")
                  for i in range(2)]
            KT = const.tile([DK, N], f32, tag="KT")
            QT = const.tile([DK, RPC], f32, tag="QT")

            # spread input loads over independent DMA queues; small weight
            # tiles first so they never queue behind the 2 MiB x loads
            nc.sync.dma_start(wk[0][:], wk_d[0:P, :])
            nc.scalar.dma_start(wk[1][:], wk_d[P:2 * P, :])
            nc.gpsimd.dma_start(wq[0][:], wq_d[0:P, :])
            nc.gpsimd.dma_start(wq[1][:], wq_d[P:2 * P, :])
            nc.sync.dma_start(xkT[0][:], xkT_d[0:P, :])
            nc.scalar.dma_start(xkT[1][:], xkT_d[P:2 * P, :])
            nc.gpsimd.dma_start(xqT[0][:], xqT_d[0:P, :])
            nc.sync.dma_start(xqT[1][:], xqT_d[P:2 * P, :])

            # Projections: KT = wk^T @ xkT, QT = wq^T @ xqT (contraction over
            # C = 256 in two accumulating halves). Only KT + the first QT
            # chunk happen up front; later QT chunks are interleaved into the
            # main loop (borrowing an S-PSUM slot) so the pipeline starts
            # ~15 us sooner.
            def proj_into(pool, tag, dst, w, src, sl):
                pt = pool.tile([DK, CHUNK], f32, tag=tag, name="pt")
                nc.tensor.matmul(out=pt[:], lhsT=mmdt(w[0][:]),
                                 rhs=mmdt(src[0][:, sl]),
                                 start=True, stop=False)
                nc.tensor.matmul(out=pt[:], lhsT=mmdt(w[1][:]),
                                 rhs=mmdt(src[1][:, sl]),
                                 start=False, stop=True)
                nc.scalar.copy(dst[:, sl], pt[:])

            with tc.tile_pool(name="proj_ps", bufs=2, space="PSUM") as proj_ps:
                for ch in range(N // CHUNK):
                    proj_into(proj_ps, "proj", KT, wk, xkT,
                              slice(ch * CHUNK, (ch + 1) * CHUNK))
                proj_into(proj_ps, "proj", QT, wq, xqT, slice(0, CHUNK))

            spool = ctx.enter_context(tc.tile_pool(name="ssb", bufs=3))
            mpool = ctx.enter_context(tc.tile_pool(name="mp", bufs=3))
            small = ctx.enter_context(tc.tile_pool(name="small", bufs=5))
            # One PSUM pool; per tile, slot a holds h0 (freed quickly by the
            # copy), slot b holds h1 (freed by the exp that reads it).
            sps = ctx.enter_context(tc.tile_pool(name="sps", bufs=2, space="PSUM"))
            MCUT = 3584   # gpsimd multiplies [0:MCUT), DVE takes the rest

            # Two-stage emission so each engine's in-order stream never
            # interleaves this tile's late ops before next tile's early ops:
            #   stage A(t):  matmuls, copy h0, max8 (h0 sbuf + h1 psum), negm
            #   stage B(t-1): tiny softmax, exp h0/h1, maskr-on-E, mult, DMA
            state = {}

            QCHUNK_TILES = CHUNK // P   # S-tiles covered per QT chunk (4)

            def stage_a(t):
                if t >= QCHUNK_TILES - 1 and (t + 1) % QCHUNK_TILES == 0:
                    nq = (t + 1) // QCHUNK_TILES   # QT chunk for tiles t+1..t+4
                    if nq < RPC // CHUNK:
                        proj_into(sps, "sps", QT, wq, xqT,
                                  slice(nq * CHUNK, (nq + 1) * CHUNK))
                s_sb = spool.tile([P, N], f32, tag="s_sb", name="s_sb")
                lhsT = QT[:, t * P:(t + 1) * P]
                pa = sps.tile([P, HALF], f32, tag="sps", name="pa")
                pb = sps.tile([P, HALF], f32, tag="sps", name="pb")
                for h, ps in ((0, pa), (1, pb)):
                    for ch in range(HALF // CHUNK):
                        psl = slice(ch * CHUNK, (ch + 1) * CHUNK)
                        ksl = slice(h * HALF + ch * CHUNK,
                                    h * HALF + (ch + 1) * CHUNK)
                        nc.tensor.matmul(out=ps[:, psl], lhsT=mmdt(lhsT),
                                         rhs=mmdt(KT[:, ksl]),
                                         start=True, stop=True)
                nc.scalar.copy(s_sb[:, 0:HALF], pa[:])

                V2 = small.tile([P, 16], f32, tag="V2", name="V2")
                nc.vector.max(V2[:, 0:8], s_sb[:, 0:HALF])
                nc.vector.max(V2[:, 8:16], pb[:])
                V = small.tile([P, 8], f32, tag="V", name="V")
                nc.vector.max(V[:], V2[:])
                negm = small.tile([P, 1], f32, tag="negm", name="negm")
                nc.vector.tensor_scalar_mul(negm[:], V[:, 0:1], -1.0)
                if k < 8:
                    nc.vector.memset(V[:, k:8], -1e30)
                state[t] = (s_sb, pb, V, negm)

            def stage_b(t):
                s_sb, pb, V, negm = state.pop(t)
                E8 = small.tile([P, 8], f32, tag="E8", name="E8")
                Z = small.tile([P, 1], f32, tag="Z", name="Z")
                nc.scalar.activation(E8[:], V[:],
                                     mybir.ActivationFunctionType.Exp,
                                     bias=negm[:, 0:1], scale=1.0,
                                     accum_out=Z[:])
                r = small.tile([P, 1], f32, tag="r", name="r")
                nc.vector.reciprocal(r[:], Z[:])

                # E = exp(S - m): h0 in place in SBUF, h1 straight from PSUM
                nc.scalar.activation(s_sb[:, 0:HALF], s_sb[:, 0:HALF],
                                     mybir.ActivationFunctionType.Exp,
                                     bias=negm[:, 0:1], scale=1.0)
                nc.scalar.activation(s_sb[:, HALF:N], pb[:],
                                     mybir.ActivationFunctionType.Exp,
                                     bias=negm[:, 0:1], scale=1.0)

                # maskr = (E >= e_k) * r  — exact same exp images on both
                # sides of the compare, so selection stays consistent. Halved
                # so the multiply can start on h0 while h1's compare runs.
                maskr = mpool.tile([P, N], f32, tag="maskr", name="maskr")
                for sl in (slice(0, HALF), slice(HALF, N)):
                    nc.vector.tensor_scalar(maskr[:, sl], s_sb[:, sl],
                                            E8[:, k - 1:k], r[:, 0:1],
                                            op0=mybir.AluOpType.is_ge,
                                            op1=mybir.AluOpType.mult)

                # A = E * maskr (in place over maskr): bulk on GpSimd in two
                # chunks, small slice on DVE at the end of its stream.
                nc.gpsimd.tensor_tensor(maskr[:, 0:HALF], s_sb[:, 0:HALF],
                                        maskr[:, 0:HALF],
                                        op=mybir.AluOpType.mult)
                nc.gpsimd.tensor_tensor(maskr[:, HALF:MCUT], s_sb[:, HALF:MCUT],
                                        maskr[:, HALF:MCUT],
                                        op=mybir.AluOpType.mult)
                nc.vector.tensor_tensor(maskr[:, MCUT:N], s_sb[:, MCUT:N],
                                        maskr[:, MCUT:N],
                                        op=mybir.AluOpType.mult)

                nc.sync.dma_start(out_d[t * P:(t + 1) * P, :], maskr[:])

            for t in range(NT + 1):
                if t < NT:
                    stage_a(t)
                if t >= 1:
                    stage_b(t - 1)

    nc.compile()
    return nc


def _get_program(k: int):
    if k not in _cache:
        _cache[k] = _build(k, f32r=F32R)
    return _cache[k]


def kernel(x, W_q, W_k, top_k):
    from concourse.bass_utils import run_bass_kernel_spmd

    x = np.asarray(x, dtype=np.float32)
    W_q = np.asarray(W_q, dtype=np.float32)
    W_k = np.asarray(W_k, dtype=np.float32)
    k = int(np.asarray(top_k))
    assert x.shape == (B, N, C) and W_q.shape == (C, DK) and W_k.shape == (C, DK)
    assert 1 <= k <= 8, f"top_k={k} unsupported"

    nc = _get_program(k)

    wq_scaled = np.ascontiguousarray(W_q * np.float32(DK) ** np.float32(-0.5),
                                     dtype=np.float32)
    wk_c = np.ascontiguousarray(W_k, dtype=np.float32)

    in_maps = []
    for c in range(NCORES):
        b, half = c // 2, c % 2
        xT = np.ascontiguousarray(x[b].T)                      # [C, N]
        xqT = np.ascontiguousarray(xT[:, half * RPC:(half + 1) * RPC])
        in_maps.append({"xkT": xT, "xqT": xqT, "wq": wq_scaled, "wk": wk_c})

    res = run_bass_kernel_spmd(nc, in_maps, list(range(NCORES)))

    A = np.empty((B, N, N), dtype=np.float32)
    for c in range(NCORES):
        b, half = c // 2, c % 2
        A[b, half * RPC:(half + 1) * RPC, :] = res.results[c]["out"]
    return A
